# revision 39
# baseline (speedup 1.0000x reference)
"""Trainium2 Bass kernel for GQA causal attention (B=2, S=2048, D=2048,
16 q-heads / 4 kv-heads, head_dim=128, interleaved RoPE).

Sharding: DP=2 over batch x TP=4 over head groups (8 cores).
Core c: batch b=c//4, rank r=c%4 -> q-heads [4r,4r+4), kv-head r.
Each core computes its heads' attention output (transposed layout [e,s]),
two column-strip AllToAlls reshard heads->sequence (overlapped with the
tail of attention), and each core runs the full output projection for its
512 strided sequence rows. Host-side work is layout only: slicing,
transposing, bf16 casting.
"""

import math
import sys

sys.path.insert(0, "/opt/trn_rl_repo")

from contextlib import ExitStack

import ml_dtypes
import numpy as np

import concourse.bass as bass
import concourse.mybir as mybir
import concourse.tile as tile
from concourse import bacc
from concourse.bass_utils import run_bass_kernel_spmd
from concourse.masks import make_identity

BF16 = mybir.dt.bfloat16
F32 = mybir.dt.float32
F32R = mybir.dt.float32r

N_HEADS = 16
N_KV_HEADS = 4
HD = 128
ROPE_THETA = 10000.0
TP = 4
N_CORES = 8


def build_graph(S=2048, D=2048, HQL=4, NS=512):
    """Per-core SPMD graph. HQL = local q heads; local kv heads = 1.

    Output ownership is strided by 128-col strips: core c owns sequence
    cols {c*128 + m*1024} of both batches; strip set m is exchanged by
    AllToAll #m as soon as the first half of attention chunks finish.
    """
    hd = HD
    ND = D // 128          # d-tiles (projection contraction tiles)
    NC = S // NS           # s-chunks
    NK = S // 128          # sk-tiles
    MQ = HQL * hd          # local q width
    DIAG = NS // 128       # sk-tiles per chunk needing a causal mask
    NB = N_CORES // TP     # batches
    OW = S // N_CORES      # out cols per core per batch
    NM = max(1, S // (N_CORES * 128))   # strips (AllToAll count)
    SW = OW // NM          # strip width (=128 at full size)
    scale = 1.0 / math.sqrt(hd)
    NH = TP * HQL          # global head count

    nc = bacc.Bacc("TRN2", target_bir_lowering=False, debug=False,
                   num_devices=N_CORES)

    xT_e = nc.dram_tensor("xT", [D, S], BF16, kind="ExternalInput").ap()
    wqT_e = nc.dram_tensor("wqT", [D, MQ], BF16, kind="ExternalInput").ap()
    wkT_e = nc.dram_tensor("wkT", [D, hd], BF16, kind="ExternalInput").ap()
    wvT_e = nc.dram_tensor("wvT", [D, hd], BF16, kind="ExternalInput").ap()
    woT_e = nc.dram_tensor("woT", [NH * hd, D], BF16,
                           kind="ExternalInput").ap()
    cc_e = nc.dram_tensor("cc", [64, S], BF16, kind="ExternalInput").ap()
    ss_e = nc.dram_tensor("ss", [64, S], BF16, kind="ExternalInput").ap()
    mask_e = nc.dram_tensor("mask", [128, NS + 384], BF16,
                            kind="ExternalInput").ap()
    out_e = nc.dram_tensor("out", [NB * OW, D], F32,
                           kind="ExternalOutput").ap()

    a2a_in = [nc.dram_tensor(f"a2a_in{m}", [N_CORES * MQ, SW], BF16)
              for m in range(NM)]
    a2a_out = [nc.dram_tensor(f"a2a_out{m}", [N_CORES * MQ, SW], BF16)
               for m in range(NM)]
    groups = [list(range(N_CORES))]

    with tile.TileContext(nc) as tc, ExitStack() as ctx:
        ep = ctx.enter_context
        const_pool = ep(tc.tile_pool(name="const", bufs=1))
        rt_pool = ep(tc.tile_pool(name="rt", bufs=HQL + 1))
        vst_pool = ep(tc.tile_pool(name="vst", bufs=1))
        pt_pool = ep(tc.tile_pool(name="pt", bufs=9))
        den_pool = ep(tc.tile_pool(name="den", bufs=4))
        recip_pool = ep(tc.tile_pool(name="recip", bufs=2))
        rbc_pool = ep(tc.tile_pool(name="rbc", bufs=2))
        attn_pool = ep(tc.tile_pool(name="attn", bufs=4))
        osb_pool = ep(tc.tile_pool(name="osb", bufs=3))
        ps_pool = ep(tc.tile_pool(name="ps", bufs=8, space="PSUM"))

        # ---- constants ----
        ident = const_pool.tile([128, 128], BF16, tag="ident")
        make_identity(nc, ident[:])
        ones = const_pool.tile([128, 32], BF16, tag="ones")
        nc.gpsimd.memset(ones[:], 1.0)
        ones_r = const_pool.tile([128, 1], F32, tag="onesr")
        nc.gpsimd.memset(ones_r[:], 1.0)
        cc = const_pool.tile([64, S], BF16, tag="cc")
        ss = const_pool.tile([64, S], BF16, tag="ss")
        msk = const_pool.tile([128, NS + 384], BF16, tag="msk")
        nc.sync.dma_start(cc[:], cc_e[:])
        nc.sync.dma_start(ss[:], ss_e[:])
        nc.sync.dma_start(msk[:], mask_e[:])

        rts = []
        vst = vst_pool.tile([128, S], BF16, tag="vst")   # vT staging
        vnat = vst_pool.tile([128, S], BF16, tag="vnat")  # v [sk, e] blocks

        # ---- phase 1: projections + rope (xt pools close after) ----
        with tc.tile_pool(name="xt", bufs=ND) as xt_pool, \
             tc.tile_pool(name="wq", bufs=ND) as wq_pool, \
             tc.tile_pool(name="wkv", bufs=2 * ND) as wkv_pool, \
             tc.tile_pool(name="tmp", bufs=4) as tmp_pool, \
             tc.tile_pool(name="stg", bufs=2) as stage_pool, \
             tc.tile_pool(name="odown", bufs=2) as odown_pool, \
             tc.tile_pool(name="ropeo", bufs=2) as ropeo_pool:
            xts, wqs, wks, wvs = [], [], [], []
            # xt+wq interleaved first (q heads project first and are
            # DMA-gated); v/k weights after
            for d in range(ND):
                xt = xt_pool.tile([128, S], BF16, tag="xt",
                                  name=f"xt{d}")
                nc.sync.dma_start(xt[:], xT_e[d * 128:(d + 1) * 128, :])
                xts.append(xt)
                wq = wq_pool.tile([128, MQ], BF16, tag="wq", name=f"wq{d}")
                nc.sync.dma_start(wq[:], wqT_e[d * 128:(d + 1) * 128, :])
                wqs.append(wq)
            for d in range(ND):
                wv = wkv_pool.tile([128, hd], BF16, tag="wkv",
                                   name=f"wv{d}")
                nc.sync.dma_start(wv[:], wvT_e[d * 128:(d + 1) * 128, :])
                wvs.append(wv)
            for d in range(ND):
                wk = wkv_pool.tile([128, hd], BF16, tag="wkv",
                                   name=f"wk{d}")
                nc.sync.dma_start(wk[:], wkT_e[d * 128:(d + 1) * 128, :])
                wks.append(wk)

            def proj_mtile(lhs_tiles, mslice, is_v, rt_tile):
                for s in range(NC):
                    ps = ps_pool.tile([128, NS], F32, tag="ps", name="psp")
                    for d in range(ND):
                        nc.tensor.matmul(
                            ps[:], lhs_tiles[d][:, mslice],
                            xts[d][:, s * NS:(s + 1) * NS],
                            start=(d == 0), stop=(d == ND - 1))
                    if is_v:
                        nc.scalar.copy(vst[:, s * NS:(s + 1) * NS], ps[:])
                    else:
                        # rope; even comps in rows 0:64, odd in 64:128
                        ssl = slice(s * NS, (s + 1) * NS)
                        stg = stage_pool.tile([128, NS], F32, tag="stg")
                        nc.scalar.copy(stg[:], ps[:])
                        od = odown_pool.tile([64, NS], F32, tag="odown")
                        nc.sync.dma_start(od[:], stg[64:128, :])
                        t0c = tmp_pool.tile([64, NS], F32, tag="tmp")
                        t1s = tmp_pool.tile([64, NS], F32, tag="tmp")
                        nc.vector.tensor_mul(t0c[:], stg[0:64, :], cc[:, ssl])
                        nc.vector.tensor_mul(t1s[:], od[:], ss[:, ssl])
                        nc.vector.tensor_sub(rt_tile[0:64, ssl],
                                             t0c[:], t1s[:])
                        t0s = tmp_pool.tile([64, NS], F32, tag="tmp")
                        t1c = tmp_pool.tile([64, NS], F32, tag="tmp")
                        nc.vector.tensor_mul(t0s[:], stg[0:64, :], ss[:, ssl])
                        nc.vector.tensor_mul(t1c[:], od[:], cc[:, ssl])
                        ro = ropeo_pool.tile([64, NS], BF16, tag="ropeo")
                        nc.vector.tensor_add(ro[:], t0s[:], t1c[:])
                        nc.sync.dma_start(rt_tile[64:128, ssl], ro[:])

            # q heads first, then v, then k LAST: attention chunk j only
            # needs k-rope of chunk j, so scores start right after k's
            # first chunk instead of after the whole projection phase
            for h in range(HQL):
                rt = rt_pool.tile([128, S], BF16, tag="rt", name=f"rtq{h}")
                proj_mtile(wqs, slice(h * hd, (h + 1) * hd), False, rt)
                rts.append(rt)
            proj_mtile(wvs, slice(0, hd), True, None)
            # v transpose: vst [e, s] -> vnat [sk, e] blocks
            for st in range(NK):
                tpp = ps_pool.tile([128, 128], BF16, tag="ps", name="pst")
                nc.tensor.transpose(
                    tpp[:], vst[:, st * 128:(st + 1) * 128], ident[:])
                nc.scalar.copy(vnat[:, st * 128:(st + 1) * 128], tpp[:])
            krt = rt_pool.tile([128, S], BF16, tag="rt", name="rtk")
            proj_mtile(wks, slice(0, hd), False, krt)

        # ---- woT preload (streams during attention; reuses xt space) ----
        wo_pool = ep(tc.tile_pool(name="wo", bufs=NH))
        ao_pool = ep(tc.tile_pool(name="ao", bufs=NB * NH * NM))
        wo_tiles = []
        for ht in range(NH):
            w = wo_pool.tile([128, D], BF16, tag="wo", name=f"wo{ht}")
            nc.sync.dma_start(w[:], woT_e[ht * 128:(ht + 1) * 128, :])
            wo_tiles.append(w)

        # ---- phase 2: attention ----
        # Head pairs per pass so psum stays within 8 banks while each
        # head's softmax denominator gets its own [1, NS] bank. The
        # score stage of each (pass, si) step is emitted one step ahead
        # of the den/av stage GLOBALLY (across pass and chunk borders),
        # so the PE never waits on the exp/mask of the current tile.
        class Pass:
            def __init__(self, j, heads):
                self.j = j
                self.heads = heads
                self.nsk = (j + 1) * DIAG
                self.at_ps = None
                self.den_ps = None

        def score_stage(p, si):
            o = si * 128 - p.j * NS
            sl = slice(p.j * NS, (p.j + 1) * NS)
            pts = {}
            for h in p.heads:
                sc = ps_pool.tile([128, NS], F32, tag="ps", name="psc")
                nc.tensor.matmul(
                    sc[:], krt[:, si * 128:(si + 1) * 128],
                    rts[h][:, sl], start=True, stop=True)
                pt = pt_pool.tile([128, NS], BF16, tag="pt")
                nc.scalar.activation(
                    pt[:], sc[:], mybir.ActivationFunctionType.Exp,
                    scale=scale)
                if o >= 0:  # diagonal block: causal mask
                    nc.vector.tensor_mul(
                        pt[:], pt[:],
                        msk[:, (NS - 128) - o:(2 * NS - 128) - o])
                pts[h] = pt
            return pts

        def denav_stage(p, si, pts):
            if p.at_ps is None:
                p.at_ps = {h: ps_pool.tile([128, NS], F32, tag="ps",
                                           name=f"atps_j{p.j}_h{h}")
                           for h in p.heads}
                # per-head softmax denominator accumulates on the DVE in
                # SBUF (f32r so the final ones-matmul runs at full PE rate)
                p.den_acc = {h: den_pool.tile([128, NS], F32, tag="den",
                                              name=f"dacc_j{p.j}_h{h}")
                             for h in p.heads}
            for h in p.heads:
                if si == 0:
                    nc.vector.tensor_copy(p.den_acc[h][:], pts[h][:])
                else:
                    nc.vector.tensor_add(p.den_acc[h][:],
                                         p.den_acc[h][:], pts[h][:])
            if si == p.nsk - 1:
                # partition-sum + recip right away so psum frees quickly
                p.rbc = {}
                for h in p.heads:
                    dbf = den_pool.tile([128, NS], BF16, tag="denb",
                                        name=f"dbf_j{p.j}_h{h}")
                    nc.vector.tensor_copy(dbf[:], p.den_acc[h][:])
                    dps = ps_pool.tile([1, NS], F32, tag="ps",
                                       name=f"dps_j{p.j}_h{h}")
                    nc.tensor.matmul(dps[:], ones[:, 0:1], dbf[:],
                                     start=True, stop=True)
                    rc = recip_pool.tile([1, NS], F32, tag="recip")
                    nc.vector.reciprocal(rc[:], dps[:])
                    rbc = rbc_pool.tile([128, NS], F32, tag="rbc")
                    nc.gpsimd.partition_broadcast(rbc[:], rc[:])
                    p.rbc[h] = rbc
            for h in p.heads:
                nc.tensor.matmul(
                    p.at_ps[h][:], vnat[:, si * 128:(si + 1) * 128],
                    pts[h][:], start=(si == 0), stop=(si == p.nsk - 1))

        def normalize_export(p):
            for h in p.heads:
                asb = attn_pool.tile([128, NS], BF16, tag="attn")
                nc.vector.tensor_mul(asb[:], p.at_ps[h][:], p.rbc[h][:])
                for i in range(NS // SW):
                    c = p.j * NS + i * SW
                    dd = (c // SW) % N_CORES
                    m = c // (N_CORES * SW)
                    nc.sync.dma_start(
                        a2a_in[m].ap()[dd * MQ + h * hd:
                                       dd * MQ + (h + 1) * hd, :],
                        asb[:, i * SW:(i + 1) * SW])

        def do_a2a(m):
            nc.gpsimd.collective_compute(
                "AllToAll", mybir.AluOpType.bypass,
                ins=[a2a_in[m].ap().opt()], outs=[a2a_out[m].ap().opt()],
                replica_groups=groups)

        per = NC // NM  # chunks per strip
        passes = []
        for j in range(NC):
            for hp in range(0, HQL, 2):
                passes.append(Pass(j, list(range(hp, min(hp + 2, HQL)))))
        pending = None
        for p in passes:
            for si in range(p.nsk):
                pts = score_stage(p, si)
                if pending is not None:
                    pp, psi, ppts = pending
                    denav_stage(pp, psi, ppts)
                    if psi == pp.nsk - 1:
                        normalize_export(pp)
                        if pp.j % per == per - 1 and \
                                pp.heads[-1] == HQL - 1:
                            do_a2a(pp.j // per)
                pending = (p, si, pts)
        pp, psi, ppts = pending
        denav_stage(pp, psi, ppts)
        normalize_export(pp)
        do_a2a(NM - 1)

        # ---- phase 3: output projection ----
        NO = D // NS
        for m in range(NM):
            for beta in range(NB):
                aos = []
                for ht in range(NH):
                    row0 = (beta * TP + ht // HQL) * MQ + (ht % HQL) * hd
                    t = ao_pool.tile([128, SW], BF16, tag="ao",
                                     name=f"ao_{m}_{beta}_{ht}")
                    nc.sync.dma_start(
                        t[:], a2a_out[m].ap()[row0:row0 + 128, :])
                    aos.append(t)
                pso = [ps_pool.tile([128, NS], F32, tag="ps",
                                    name=f"pso_{m}_{beta}_{n}")
                       for n in range(NO)]
                for ht in range(NH):
                    for n in range(NO):
                        nc.tensor.matmul(
                            pso[n][:], aos[ht][:],
                            wo_tiles[ht][:, n * NS:(n + 1) * NS],
                            start=(ht == 0), stop=(ht == NH - 1))
                r0 = beta * OW + m * SW
                for n in range(NO):
                    ob = osb_pool.tile([128, NS], F32, tag="osb")
                    nc.scalar.copy(ob[:], pso[n][:])
                    nc.sync.dma_start(
                        out_e[r0:r0 + SW, n * NS:(n + 1) * NS], ob[0:SW, :])

    nc.compile()
    return nc


def host_prepare(x, wq, wk, wv, wo, S, D, HQL, NS):
    """Layout-only host prep: slice/transpose/cast + rope tables + mask."""
    hd = HD
    MQ = HQL * hd
    bf = ml_dtypes.bfloat16

    perm = np.concatenate([np.arange(0, hd, 2), np.arange(1, hd, 2)])

    def permute_heads(w):
        nh = w.shape[0] // hd
        w = w.reshape(nh, hd, -1)[:, perm, :]
        return w.reshape(nh * hd, -1)

    wq_p = permute_heads(wq)
    wk_p = permute_heads(wk)

    inv_freq = 1.0 / (ROPE_THETA ** (np.arange(0, hd, 2, dtype=np.float64)
                                     / hd))
    ang = np.arange(S, dtype=np.float64)[None, :] * inv_freq[:, None]
    cci = np.cos(ang).astype(bf)
    ssi = np.sin(ang).astype(bf)

    p = np.arange(128)[:, None]
    c = np.arange(NS + 384)[None, :]
    mski = (p <= c - (NS - 128)).astype(bf)

    woT = np.ascontiguousarray(wo.T).astype(bf)

    in_maps = []
    for core in range(N_CORES):
        b = core // TP
        r = core % TP
        qsl = slice(r * MQ, (r + 1) * MQ)
        ksl = slice(r * hd, (r + 1) * hd)
        in_maps.append({
            "xT": np.ascontiguousarray(x[b].T).astype(bf),
            "wqT": np.ascontiguousarray(wq_p[qsl].T).astype(bf),
            "wkT": np.ascontiguousarray(wk_p[ksl].T).astype(bf),
            "wvT": np.ascontiguousarray(wv[ksl].T).astype(bf),
            "woT": woT,
            "cc": cci, "ss": ssi, "mask": mski,
        })
    return in_maps


_NC_CACHE = {}


def get_graph(S=2048, D=2048, HQL=4, NS=512):
    key = (S, D, HQL, NS)
    if key not in _NC_CACHE:
        _NC_CACHE[key] = build_graph(S, D, HQL, NS)
    return _NC_CACHE[key]


def unshard_out(results, B, S, D):
    """results[core]["out"] is [NB*OW, D] with rows (beta, strip m, 128)."""
    out = np.empty((B, S, D), dtype=np.float32)
    OW = S // N_CORES
    NM = max(1, S // (N_CORES * 128))
    SW = OW // NM
    for core in range(N_CORES):
        r = results[core]["out"]
        for beta in range(B):
            for m in range(NM):
                c0 = core * SW + m * N_CORES * SW
                out[beta, c0:c0 + SW, :] = \
                    r[beta * OW + m * SW:beta * OW + (m + 1) * SW, :]
    return out


def kernel(x, wq, wk, wv, wo, trace=False):
    B, S, D = x.shape
    HQL = (wq.shape[0] // HD) // TP
    NS = 512
    nc = get_graph(S, D, HQL, NS)
    in_maps = host_prepare(x, wq, wk, wv, wo, S, D, HQL, NS)
    res = run_bass_kernel_spmd(nc, in_maps, core_ids=list(range(N_CORES)),
                               trace=trace)
    out = unshard_out(res.results, B, S, D)
    if trace:
        kernel.last_exec_time_ns = res.exec_time_ns
        kernel.last_results = res
    return out
